# revision 21
# baseline (speedup 1.0000x reference)
"""NearAggregator Trainium2 Bass kernel.

Math (per batch item b):
    Kcat   = concat([near_emb, delta_xy, delta_cs], -1)          # [N, 132]
    scores = (Kcat @ W_key + b_key) . B_query[b] / sqrt(64)      # [N]
    out[b] = softmax(scores) @ near_emb[b]                       # [128]

Reformulation:
  * Fold W_key into the query side:  qp[b,:] = 0.125 * (W_key^T @ B_query[b])
    (132-dim), so scores[b,n] = near[b,n,:].qp[b,:128] + delta[b,n,:].qp[b,128:132].
  * b_key only shifts scores by a per-b constant -> softmax-invariant -> dropped.
  * softmax without max-subtraction: scores ~ N(0, 0.58), exp() safe in fp32.

Pipeline (per 128-item group g, engines in parallel):
  * DVE computes scores via fused fp32 mult+reduce (near_n . qp) per
    neighbor; delta contribution via 4 fp32 ops; exp per 16-neighbor chunk
    on ScalarE (chunk-local softmax needs no max subtraction); 1/sumexp on
    DVE once per group.
  * scaled_n = near_n * e[:,n] -> bf16 (inline convert), split
    ScalarE/DVE by a tunable mix. Four neighbors' scaled tiles share one
    [G, 4, D] quad tile.
  * TensorE accumulates each quad with a single matmul (identity_bf16
    stationary, 512-wide moving) into a [G, 4, D] PSUM accumulator:
    4x fewer PE instructions, no per-neighbor weight reload stalls.
  * Epilogue (deferred one group): sum the 4 PSUM bands + multiply by
    1/sumexp on DVE, store.

Data parallel over 8 NeuronCores: batch 8192 -> 1024 per core.
"""

import os

import numpy as np

B = 8192
N = 128
D = 128
DQ = 64
F = D + 4
CORES = 8
PB = B // CORES            # 1024 items per core
G = 128                    # items per group (= partition dim)
NGROUPS = PB // G          # 8
NCH = 16                   # neighbors per near tile
NT = N // NCH              # 8 tiles per group
NQ = 4                     # neighbors per matmul quad


# Per-neighbor scale-op engine mix: a=ScalarE, v=DVE (bf16 fast path).
def _mix_pattern(spec):
    parts = [(k, int(v)) for k, v in (p.split(":") for p in spec.split(","))]
    total = sum(c for _, c in parts)
    pat = []
    err = [0.0] * len(parts)
    for _ in range(total):
        for i in range(len(parts)):
            err[i] += parts[i][1] / total
        best = max(range(len(parts)), key=lambda i: err[i])
        err[best] -= 1.0
        pat.append(parts[best][0])
    return "".join(pat)


SCALE_PATTERN = _mix_pattern(os.environ.get("NK_MIX", "a:18,v:7"))

_NC = None


def _build():
    import concourse.tile as tile
    from concourse import bacc, mybir

    f32 = mybir.dt.float32
    bf16 = mybir.dt.bfloat16
    mult = mybir.AluOpType.mult
    add = mybir.AluOpType.add
    bypass = mybir.AluOpType.bypass

    nc = bacc.Bacc(
        "TRN2",
        target_bir_lowering=False,
        debug=False,
        enable_asserts=True,
        num_devices=CORES,
    )
    near = nc.dram_tensor("near", [PB, N, D], f32, kind="ExternalInput").ap()
    dxy = nc.dram_tensor("dxy", [PB, N, 2], f32, kind="ExternalInput").ap()
    dcs = nc.dram_tensor("dcs", [PB, N, 2], f32, kind="ExternalInput").ap()
    bq = nc.dram_tensor("bq", [PB, DQ], f32, kind="ExternalInput").ap()
    wk = nc.dram_tensor("wk", [F, DQ], f32, kind="ExternalInput").ap()
    out = nc.dram_tensor("out", [PB, D], f32, kind="ExternalOutput").ap()
    ident_dram = nc.inline_tensor(np.eye(128, dtype=np.float32), name="ident").ap()

    pattern = SCALE_PATTERN

    with tile.TileContext(nc) as tc:
        from contextlib import ExitStack

        ctx = ExitStack()
        with ctx:
            consts = ctx.enter_context(tc.tile_pool(name="consts", bufs=1))
            nearf = ctx.enter_context(tc.tile_pool(name="nearf", bufs=2 * NT))
            dlp = ctx.enter_context(tc.tile_pool(name="dlp", bufs=4))
            smq = ctx.enter_context(tc.tile_pool(name="smq", bufs=4))
            qpp = ctx.enter_context(tc.tile_pool(name="qpp", bufs=2))
            scp = ctx.enter_context(tc.tile_pool(name="scp", bufs=2))
            scratch = ctx.enter_context(tc.tile_pool(name="scratch", bufs=4))
            scaledp = ctx.enter_context(tc.tile_pool(name="scaledp", bufs=4))
            outp = ctx.enter_context(tc.tile_pool(name="outp", bufs=2))
            psp = ctx.enter_context(tc.tile_pool(name="psp", bufs=2, space="PSUM"))
            psq = ctx.enter_context(tc.tile_pool(name="psq", bufs=3, space="PSUM"))
            pss = ctx.enter_context(tc.tile_pool(name="pss", bufs=1, space="PSUM"))

            # ---- one-time setup ----
            identity = consts.tile([128, 128], f32)
            nc.sync.dma_start(identity[:], ident_dram[:])
            id_bf = consts.tile([128, 128], bf16)
            nc.scalar.copy(id_bf[:], identity[:])

            # wT = 0.125 * W_key^T  as [64, 132]
            w1 = consts.tile([128, DQ], f32)
            nc.sync.dma_start(w1[:], wk[0:128, :])
            w2 = consts.tile([4, DQ], f32)
            nc.sync.dma_start(w2[:], wk[128:132, :])
            wT = consts.tile([DQ, F], f32)
            stp = pss.tile([DQ, 128], f32, tag="setup_ps")
            nc.tensor.transpose(stp[:], w1[:], identity[:])
            nc.scalar.mul(wT[:, 0:128], stp[:], 0.125)
            stp2 = pss.tile([DQ, 4], f32, tag="setup_ps")
            nc.tensor.transpose(stp2[:], w2[:], identity[0:4, 0:4])
            nc.scalar.mul(wT[:, 128:132], stp2[:], 0.125)

            def emit_loads(gi):
                """DMA fp32 near chunks + delta tensors for group gi."""
                b0 = gi * G
                dxy_t = dlp.tile([G, N, 2], f32, tag="dl")
                nc.sync.dma_start(dxy_t[:], dxy[b0 : b0 + G, :, :])
                dcs_t = dlp.tile([G, N, 2], f32, tag="dl")
                nc.sync.dma_start(dcs_t[:], dcs[b0 : b0 + G, :, :])
                nmf = []
                for c in range(NT):
                    t = nearf.tile([G, NCH, D], f32, name=f"nf{gi}_{c}", tag="nf")
                    nc.sync.dma_start(
                        t[:], near[b0 : b0 + G, c * NCH : (c + 1) * NCH, :]
                    )
                    nmf.append(t)
                return dxy_t, dcs_t, nmf

            def emit_qp(gi):
                b0 = gi * G
                bq_t = smq.tile([G, DQ], f32, tag="sm")
                nc.sync.dma_start(bq_t[:], bq[b0 : b0 + G, :])
                bqT_ps = psq.tile([DQ, G], f32, tag="qpps")
                nc.tensor.transpose(bqT_ps[:], bq_t[:], identity[:])
                bqT = smq.tile([DQ, G], f32, tag="sm")
                nc.scalar.copy(bqT[:], bqT_ps[:])
                qp_ps = psq.tile([G, F], f32, tag="qpps")
                nc.tensor.matmul(qp_ps[:], bqT[:], wT[:], start=True, stop=True)
                qp = qpp.tile([G, F], f32, tag="qp")
                nc.scalar.copy(qp[:], qp_ps[:])
                return qp

            loads = emit_loads(0)
            qp = emit_qp(0)
            pending = None

            def emit_epilogue(p_pooled4, p_recip, p_b0):
                # one PSUM operand per op (hw limit); chain adds through SBUF
                a0 = scratch.tile([G, D], f32, tag="a0")
                nc.vector.tensor_copy(a0[:], p_pooled4[:, 0, :])
                a1 = scratch.tile([G, D], f32, tag="a1")
                nc.vector.tensor_tensor(a1[:], p_pooled4[:, 1, :], a0[:], op=add)
                a2 = scratch.tile([G, D], f32, tag="a2")
                nc.vector.tensor_tensor(a2[:], p_pooled4[:, 2, :], a1[:], op=add)
                a3 = scratch.tile([G, D], f32, tag="a3")
                nc.vector.tensor_tensor(a3[:], p_pooled4[:, 3, :], a2[:], op=add)
                out_t = outp.tile([G, D], f32, tag="out")
                nc.vector.tensor_scalar_mul(out_t[:], a3[:], p_recip[:])
                nc.sync.dma_start(out[p_b0 : p_b0 + G, :], out_t[:])

            for gi in range(NGROUPS):
                b0 = gi * G
                dxy_t, dcs_t, nmf = loads
                if gi + 1 < NGROUPS:
                    loads = emit_loads(gi + 1)
                    qp_next = emit_qp(gi + 1)
                else:
                    qp_next = None

                # ---- delta score contribution sc4[g, n] (DVE, fp32) ----
                s1 = scp.tile([G, N], f32, tag="s1")
                nc.vector.tensor_scalar_mul(s1[:], dxy_t[:, :, 0], qp[:, 128:129])
                s2 = scp.tile([G, N], f32, tag="s2")
                nc.vector.scalar_tensor_tensor(
                    s2[:], dxy_t[:, :, 1], qp[:, 129:130], s1[:], op0=mult, op1=add
                )
                s3 = scp.tile([G, N], f32, tag="s3")
                nc.vector.scalar_tensor_tensor(
                    s3[:], dcs_t[:, :, 0], qp[:, 130:131], s2[:], op0=mult, op1=add
                )
                sc4 = scp.tile([G, N], f32, tag="sc4")
                nc.vector.scalar_tensor_tensor(
                    sc4[:], dcs_t[:, :, 1], qp[:, 131:132], s3[:], op0=mult, op1=add
                )

                # ---- chunk-local: scores -> exp -> bf16 quads -> matmul ----
                scores0 = scp.tile([G, N], f32, tag="scores0")
                scsum = scp.tile([G, N], f32, tag="scsum")
                e_t = scp.tile([G, N], f32, tag="et")
                pooled4 = psp.tile([G, NQ, D], f32, tag="pool")
                qpc = NCH // NQ   # quads per chunk
                for c in range(NT):
                    cs = slice(c * NCH, (c + 1) * NCH)
                    for j in range(NCH):
                        n = c * NCH + j
                        pr = scratch.tile([G, D], f32, name=f"pr{n}", tag="pr")
                        nc.vector.scalar_tensor_tensor(
                            out=pr[:],
                            in0=nmf[c][:, j, :],
                            scalar=1.0,
                            in1=qp[:, 0:D],
                            op0=bypass,
                            op1=mult,
                            accum_out=scores0[:, n : n + 1],
                        )
                    nc.vector.tensor_tensor(
                        scsum[:, cs], scores0[:, cs], sc4[:, cs], op=add
                    )
                    nc.scalar.activation(
                        e_t[:, cs],
                        scsum[:, cs],
                        func=mybir.ActivationFunctionType.Exp,
                    )
                    for qq in range(qpc):
                        q = c * qpc + qq
                        quad = scaledp.tile([G, NQ, D], bf16, name=f"qd{q}", tag="qd")
                        for k in range(NQ):
                            n = q * NQ + k
                            j = n - c * NCH
                            eng = pattern[n % len(pattern)]
                            eap = e_t[:, n : n + 1]
                            if eng == "a":
                                nc.scalar.mul(quad[:, k, :], nmf[c][:, j, :], eap)
                            else:
                                nc.vector.tensor_scalar_mul(
                                    quad[:, k, :], nmf[c][:, j, :], eap
                                )
                        nc.tensor.matmul(
                            pooled4[:],
                            id_bf[:],
                            quad[:],
                            start=(q == 0),
                            stop=(q == N // NQ - 1),
                        )

                # ---- sumexp, reciprocal ----
                sume = scp.tile([G, 1], f32, tag="sume")
                nc.vector.tensor_reduce(
                    out=sume[:], in_=e_t[:], axis=mybir.AxisListType.X, op=add
                )
                recip = scp.tile([G, 1], f32, tag="recip")
                nc.vector.reciprocal(recip[:], sume[:])

                # ---- deferred epilogue of previous group ----
                if pending is not None:
                    emit_epilogue(*pending)
                pending = (pooled4, recip, b0)
                qp = qp_next

            emit_epilogue(*pending)

    nc.compile()
    return nc


def _get_nc():
    global _NC
    if _NC is None:
        _NC = _build()
    return _NC


def kernel(near_emb, delta_xy, delta_cs, B_query, W_key, b_key=None, **_ignored):
    from concourse import bass_utils

    near_emb = np.ascontiguousarray(np.asarray(near_emb, dtype=np.float32))
    delta_xy = np.ascontiguousarray(np.asarray(delta_xy, dtype=np.float32))
    delta_cs = np.ascontiguousarray(np.asarray(delta_cs, dtype=np.float32))
    B_query = np.ascontiguousarray(np.asarray(B_query, dtype=np.float32))
    W_key = np.ascontiguousarray(np.asarray(W_key, dtype=np.float32))

    nc = _get_nc()
    in_maps = []
    for c in range(CORES):
        s = slice(c * PB, (c + 1) * PB)
        in_maps.append(
            {
                "near": near_emb[s],
                "dxy": delta_xy[s],
                "dcs": delta_cs[s],
                "bq": B_query[s],
                "wk": W_key,
            }
        )
    res = bass_utils.run_bass_kernel_spmd(nc, in_maps, core_ids=list(range(CORES)))
    return np.concatenate([res.results[c]["out"] for c in range(CORES)], axis=0)


# revision 24
# speedup vs baseline: 1.0927x; 1.0927x over previous
"""NearAggregator Trainium2 Bass kernel.

Math (per batch item b):
    Kcat   = concat([near_emb, delta_xy, delta_cs], -1)          # [N, 132]
    scores = (Kcat @ W_key + b_key) . B_query[b] / sqrt(64)      # [N]
    out[b] = softmax(scores) @ near_emb[b]                       # [128]

Reformulation:
  * Fold W_key into the query side:  qp[b,:] = 0.125 * (W_key^T @ B_query[b])
    (132-dim), so scores[b,n] = near[b,n,:].qp[b,:128] + delta[b,n,:].qp[b,128:132].
  * b_key only shifts scores by a per-b constant -> softmax-invariant -> dropped.
  * softmax without max-subtraction: scores ~ N(0, 0.58), exp() safe in fp32.

Pipeline (per 128-item group g, engines in parallel):
  * DVE computes scores via fused fp32 mult+reduce (near_n . qp) per
    neighbor; delta contribution via 4 fp32 ops; exp per 16-neighbor chunk
    on ScalarE (chunk-local softmax needs no max subtraction); 1/sumexp on
    DVE once per group.
  * scaled_n = near_n * e[:,n] -> bf16 (inline convert), split
    ScalarE/DVE by a tunable mix. Four neighbors' scaled tiles share one
    [G, 4, D] quad tile.
  * TensorE accumulates each quad with a single matmul (identity_bf16
    stationary, 512-wide moving) into a [G, 4, D] PSUM accumulator:
    4x fewer PE instructions, no per-neighbor weight reload stalls.
  * Epilogue (deferred one group): sum the 4 PSUM bands + multiply by
    1/sumexp on DVE, store.

Data parallel over 8 NeuronCores: batch 8192 -> 1024 per core.
"""

import os

import numpy as np

B = 8192
N = 128
D = 128
DQ = 64
F = D + 4
CORES = 8
PB = B // CORES            # 1024 items per core
G = 128                    # items per group (= partition dim)
NGROUPS = PB // G          # 8
NCH = 16                   # neighbors per near tile
NT = N // NCH              # 8 tiles per group
NQ = 2                     # neighbors per matmul quad


# Per-neighbor scale-op engine mix: a=ScalarE, v=DVE (bf16 fast path).
def _mix_pattern(spec):
    parts = [(k, int(v)) for k, v in (p.split(":") for p in spec.split(","))]
    total = sum(c for _, c in parts)
    pat = []
    err = [0.0] * len(parts)
    for _ in range(total):
        for i in range(len(parts)):
            err[i] += parts[i][1] / total
        best = max(range(len(parts)), key=lambda i: err[i])
        err[best] -= 1.0
        pat.append(parts[best][0])
    return "".join(pat)


SCALE_PATTERN = _mix_pattern(os.environ.get("NK_MIX", "a:18,v:7"))

_NC = None


def _build():
    import concourse.tile as tile
    from concourse import bacc, mybir

    f32 = mybir.dt.float32
    bf16 = mybir.dt.bfloat16
    mult = mybir.AluOpType.mult
    add = mybir.AluOpType.add
    bypass = mybir.AluOpType.bypass

    nc = bacc.Bacc(
        "TRN2",
        target_bir_lowering=False,
        debug=False,
        enable_asserts=True,
        num_devices=CORES,
    )
    near = nc.dram_tensor("near", [PB, N, D], f32, kind="ExternalInput").ap()
    dxy = nc.dram_tensor("dxy", [PB, N, 2], f32, kind="ExternalInput").ap()
    dcs = nc.dram_tensor("dcs", [PB, N, 2], f32, kind="ExternalInput").ap()
    bq = nc.dram_tensor("bq", [PB, DQ], f32, kind="ExternalInput").ap()
    wk = nc.dram_tensor("wk", [F, DQ], f32, kind="ExternalInput").ap()
    out = nc.dram_tensor("out", [PB, D], f32, kind="ExternalOutput").ap()
    ident_dram = nc.inline_tensor(np.eye(128, dtype=np.float32), name="ident").ap()

    pattern = SCALE_PATTERN

    with tile.TileContext(nc) as tc:
        from contextlib import ExitStack

        ctx = ExitStack()
        with ctx:
            consts = ctx.enter_context(tc.tile_pool(name="consts", bufs=1))
            nearf = ctx.enter_context(tc.tile_pool(name="nearf", bufs=2 * NT + 1))
            dlp = ctx.enter_context(tc.tile_pool(name="dlp", bufs=4))
            smq = ctx.enter_context(tc.tile_pool(name="smq", bufs=4))
            qpp = ctx.enter_context(tc.tile_pool(name="qpp", bufs=2))
            scp = ctx.enter_context(tc.tile_pool(name="scp", bufs=2))
            scratch = ctx.enter_context(tc.tile_pool(name="scratch", bufs=4))
            scaledp = ctx.enter_context(tc.tile_pool(name="scaledp", bufs=8))
            outp = ctx.enter_context(tc.tile_pool(name="outp", bufs=2))
            psp = ctx.enter_context(tc.tile_pool(name="psp", bufs=2, space="PSUM"))
            psq = ctx.enter_context(tc.tile_pool(name="psq", bufs=3, space="PSUM"))
            pss = ctx.enter_context(tc.tile_pool(name="pss", bufs=1, space="PSUM"))

            # ---- one-time setup ----
            identity = consts.tile([128, 128], f32)
            nc.sync.dma_start(identity[:], ident_dram[:])
            id_bf = consts.tile([128, 128], bf16)
            nc.scalar.copy(id_bf[:], identity[:])

            # wT = 0.125 * W_key^T  as [64, 132]
            w1 = consts.tile([128, DQ], f32)
            nc.sync.dma_start(w1[:], wk[0:128, :])
            w2 = consts.tile([4, DQ], f32)
            nc.sync.dma_start(w2[:], wk[128:132, :])
            wT = consts.tile([DQ, F], f32)
            stp = pss.tile([DQ, 128], f32, tag="setup_ps")
            nc.tensor.transpose(stp[:], w1[:], identity[:])
            nc.scalar.mul(wT[:, 0:128], stp[:], 0.125)
            stp2 = pss.tile([DQ, 4], f32, tag="setup_ps")
            nc.tensor.transpose(stp2[:], w2[:], identity[0:4, 0:4])
            nc.scalar.mul(wT[:, 128:132], stp2[:], 0.125)

            def emit_loads(gi):
                """DMA fp32 near chunks + delta tensors for group gi."""
                b0 = gi * G
                dxy_t = dlp.tile([G, N, 2], f32, tag="dl")
                nc.sync.dma_start(dxy_t[:], dxy[b0 : b0 + G, :, :])
                dcs_t = dlp.tile([G, N, 2], f32, tag="dl")
                nc.sync.dma_start(dcs_t[:], dcs[b0 : b0 + G, :, :])
                nmf = []
                for c in range(NT):
                    t = nearf.tile([G, NCH, D], f32, name=f"nf{gi}_{c}", tag="nf")
                    nc.sync.dma_start(
                        t[:], near[b0 : b0 + G, c * NCH : (c + 1) * NCH, :]
                    )
                    nmf.append(t)
                return dxy_t, dcs_t, nmf

            def emit_qp(gi):
                b0 = gi * G
                bq_t = smq.tile([G, DQ], f32, tag="sm")
                nc.sync.dma_start(bq_t[:], bq[b0 : b0 + G, :])
                bqT_ps = psq.tile([DQ, G], f32, tag="qpps")
                nc.tensor.transpose(bqT_ps[:], bq_t[:], identity[:])
                bqT = smq.tile([DQ, G], f32, tag="sm")
                nc.scalar.copy(bqT[:], bqT_ps[:])
                qp_ps = psq.tile([G, F], f32, tag="qpps")
                nc.tensor.matmul(qp_ps[:], bqT[:], wT[:], start=True, stop=True)
                qp = qpp.tile([G, F], f32, tag="qp")
                nc.scalar.copy(qp[:], qp_ps[:])
                return qp

            qp = emit_qp(0)
            loads = emit_loads(0)
            pending = None

            def emit_epilogue(p_pooled4, p_recip, p_b0):
                # out = (band0 + band1) / sumexp, one PSUM operand per op:
                # a0 = band0 * recip; out = (band1 * recip) + a0
                a0 = scratch.tile([G, D], f32, tag="a0")
                nc.vector.tensor_scalar_mul(a0[:], p_pooled4[:, 0, :], p_recip[:])
                out_t = outp.tile([G, D], f32, tag="out")
                nc.vector.scalar_tensor_tensor(
                    out=out_t[:],
                    in0=p_pooled4[:, 1, :],
                    scalar=p_recip[:],
                    in1=a0[:],
                    op0=mult,
                    op1=add,
                )
                nc.sync.dma_start(out[p_b0 : p_b0 + G, :], out_t[:])

            for gi in range(NGROUPS):
                b0 = gi * G
                dxy_t, dcs_t, nmf = loads
                if gi + 1 < NGROUPS:
                    qp_next = emit_qp(gi + 1)
                    loads = emit_loads(gi + 1)
                else:
                    qp_next = None

                # ---- delta score contribution sc4[g, n] (DVE, fp32) ----
                s1 = scp.tile([G, N], f32, tag="s1")
                nc.vector.tensor_scalar_mul(s1[:], dxy_t[:, :, 0], qp[:, 128:129])
                s2 = scp.tile([G, N], f32, tag="s2")
                nc.vector.scalar_tensor_tensor(
                    s2[:], dxy_t[:, :, 1], qp[:, 129:130], s1[:], op0=mult, op1=add
                )
                s3 = scp.tile([G, N], f32, tag="s3")
                nc.vector.scalar_tensor_tensor(
                    s3[:], dcs_t[:, :, 0], qp[:, 130:131], s2[:], op0=mult, op1=add
                )
                sc4 = scp.tile([G, N], f32, tag="sc4")
                nc.vector.scalar_tensor_tensor(
                    sc4[:], dcs_t[:, :, 1], qp[:, 131:132], s3[:], op0=mult, op1=add
                )

                # ---- segmented: scores -> exp -> bf16 quads -> matmul ----
                # Fine segments for the very first block (pipeline fill);
                # half-group segments otherwise (fewer, wider add/exp ops).
                scores0 = scp.tile([G, N], f32, tag="scores0")
                scsum = scp.tile([G, N], f32, tag="scsum")
                e_t = scp.tile([G, N], f32, tag="et")
                pooled4 = psp.tile([G, NQ, D], f32, tag="pool")
                if gi == 0:
                    segs = [(0, 16), (16, 32), (32, 48), (48, 64),
                            (64, 96), (96, 128)]
                elif gi == 1:
                    segs = [(0, 32), (32, 64), (64, 128)]
                else:
                    segs = [(0, 64), (64, 128)]
                for n0, n1 in segs:
                    ss = slice(n0, n1)
                    for n in range(n0, n1):
                        c, j = divmod(n, NCH)
                        pr = scratch.tile([G, D], f32, name=f"pr{n}", tag="pr")
                        nc.vector.scalar_tensor_tensor(
                            out=pr[:],
                            in0=nmf[c][:, j, :],
                            scalar=1.0,
                            in1=qp[:, 0:D],
                            op0=bypass,
                            op1=mult,
                            accum_out=scores0[:, n : n + 1],
                        )
                    nc.vector.tensor_tensor(
                        scsum[:, ss], scores0[:, ss], sc4[:, ss], op=add
                    )
                    nc.scalar.activation(
                        e_t[:, ss],
                        scsum[:, ss],
                        func=mybir.ActivationFunctionType.Exp,
                    )
                    for q in range(n0 // NQ, n1 // NQ):
                        quad = scaledp.tile([G, NQ, D], bf16, name=f"qd{q}", tag="qd")
                        for k in range(NQ):
                            n = q * NQ + k
                            c, j = divmod(n, NCH)
                            eng = pattern[n % len(pattern)]
                            eap = e_t[:, n : n + 1]
                            if eng == "a":
                                nc.scalar.mul(quad[:, k, :], nmf[c][:, j, :], eap)
                            else:
                                nc.vector.tensor_scalar_mul(
                                    quad[:, k, :], nmf[c][:, j, :], eap
                                )
                        nc.tensor.matmul(
                            pooled4[:],
                            id_bf[:],
                            quad[:],
                            start=(q == 0),
                            stop=(q == N // NQ - 1),
                        )

                # ---- sumexp, reciprocal ----
                sume = scp.tile([G, 1], f32, tag="sume")
                nc.vector.tensor_reduce(
                    out=sume[:], in_=e_t[:], axis=mybir.AxisListType.X, op=add
                )
                recip = scp.tile([G, 1], f32, tag="recip")
                nc.vector.reciprocal(recip[:], sume[:])

                # ---- deferred epilogue of previous group ----
                if pending is not None:
                    emit_epilogue(*pending)
                pending = (pooled4, recip, b0)
                qp = qp_next

            emit_epilogue(*pending)

    nc.compile()
    return nc


def _get_nc():
    global _NC
    if _NC is None:
        _NC = _build()
    return _NC


def kernel(near_emb, delta_xy, delta_cs, B_query, W_key, b_key=None, **_ignored):
    from concourse import bass_utils

    near_emb = np.ascontiguousarray(np.asarray(near_emb, dtype=np.float32))
    delta_xy = np.ascontiguousarray(np.asarray(delta_xy, dtype=np.float32))
    delta_cs = np.ascontiguousarray(np.asarray(delta_cs, dtype=np.float32))
    B_query = np.ascontiguousarray(np.asarray(B_query, dtype=np.float32))
    W_key = np.ascontiguousarray(np.asarray(W_key, dtype=np.float32))

    nc = _get_nc()
    in_maps = []
    for c in range(CORES):
        s = slice(c * PB, (c + 1) * PB)
        in_maps.append(
            {
                "near": near_emb[s],
                "dxy": delta_xy[s],
                "dcs": delta_cs[s],
                "bq": B_query[s],
                "wk": W_key,
            }
        )
    res = bass_utils.run_bass_kernel_spmd(nc, in_maps, core_ids=list(range(CORES)))
    return np.concatenate([res.results[c]["out"] for c in range(CORES)], axis=0)


# revision 29
# speedup vs baseline: 1.0941x; 1.0013x over previous
"""NearAggregator Trainium2 Bass kernel.

Math (per batch item b):
    Kcat   = concat([near_emb, delta_xy, delta_cs], -1)          # [N, 132]
    scores = (Kcat @ W_key + b_key) . B_query[b] / sqrt(64)      # [N]
    out[b] = softmax(scores) @ near_emb[b]                       # [128]

Reformulation:
  * Fold W_key into the query side:  qp[b,:] = 0.125 * (W_key^T @ B_query[b])
    (132-dim), so scores[b,n] = near[b,n,:].qp[b,:128] + delta[b,n,:].qp[b,128:132].
  * b_key only shifts scores by a per-b constant -> softmax-invariant -> dropped.
  * softmax without max-subtraction: scores ~ N(0, 0.58), exp() safe in fp32.

Pipeline (per 128-item group g, engines in parallel):
  * DVE computes scores via fused fp32 mult+reduce (near_n . qp) per
    neighbor; delta contribution via 4 fp32 ops; exp per 16-neighbor chunk
    on ScalarE (chunk-local softmax needs no max subtraction); 1/sumexp on
    DVE once per group.
  * scaled_n = near_n * e[:,n] -> bf16 (inline convert), split
    ScalarE/DVE by a tunable mix. Four neighbors' scaled tiles share one
    [G, 4, D] quad tile.
  * TensorE accumulates each quad with a single matmul (identity_bf16
    stationary, 512-wide moving) into a [G, 4, D] PSUM accumulator:
    4x fewer PE instructions, no per-neighbor weight reload stalls.
  * Epilogue (deferred one group): sum the 4 PSUM bands + multiply by
    1/sumexp on DVE, store.

Data parallel over 8 NeuronCores: batch 8192 -> 1024 per core.
"""

import os

import numpy as np

B = 8192
N = 128
D = 128
DQ = 64
F = D + 4
CORES = 8
PB = B // CORES            # 1024 items per core
G = 128                    # items per group (= partition dim)
NGROUPS = PB // G          # 8
NCH = 16                   # neighbors per near tile
NT = N // NCH              # 8 tiles per group
NQ = 2                     # neighbors per matmul quad


# Per-neighbor scale-op engine mix: a=ScalarE, v=DVE (bf16 fast path).
def _mix_pattern(spec):
    parts = [(k, int(v)) for k, v in (p.split(":") for p in spec.split(","))]
    total = sum(c for _, c in parts)
    pat = []
    err = [0.0] * len(parts)
    for _ in range(total):
        for i in range(len(parts)):
            err[i] += parts[i][1] / total
        best = max(range(len(parts)), key=lambda i: err[i])
        err[best] -= 1.0
        pat.append(parts[best][0])
    return "".join(pat)


SCALE_PATTERN = _mix_pattern(os.environ.get("NK_MIX", "a:18,v:7"))

_NC = None


def _build():
    import concourse.tile as tile
    from concourse import bacc, mybir

    f32 = mybir.dt.float32
    bf16 = mybir.dt.bfloat16
    mult = mybir.AluOpType.mult
    add = mybir.AluOpType.add
    bypass = mybir.AluOpType.bypass

    nc = bacc.Bacc(
        "TRN2",
        target_bir_lowering=False,
        debug=False,
        enable_asserts=True,
        num_devices=CORES,
    )
    near = nc.dram_tensor("near", [PB, N, D], f32, kind="ExternalInput").ap()
    dxy = nc.dram_tensor("dxy", [PB, N, 2], f32, kind="ExternalInput").ap()
    dcs = nc.dram_tensor("dcs", [PB, N, 2], f32, kind="ExternalInput").ap()
    bq = nc.dram_tensor("bq", [PB, DQ], f32, kind="ExternalInput").ap()
    wk = nc.dram_tensor("wk", [F, DQ], f32, kind="ExternalInput").ap()
    out = nc.dram_tensor("out", [PB, D], f32, kind="ExternalOutput").ap()
    ident_dram = nc.inline_tensor(np.eye(128, dtype=np.float32), name="ident").ap()

    pattern = SCALE_PATTERN

    with tile.TileContext(nc) as tc:
        from contextlib import ExitStack

        ctx = ExitStack()
        with ctx:
            consts = ctx.enter_context(tc.tile_pool(name="consts", bufs=1))
            nearf = ctx.enter_context(tc.tile_pool(name="nearf", bufs=2 * NT + 1))
            dlp = ctx.enter_context(tc.tile_pool(name="dlp", bufs=4))
            smq = ctx.enter_context(tc.tile_pool(name="smq", bufs=4))
            qpp = ctx.enter_context(tc.tile_pool(name="qpp", bufs=2))
            scp = ctx.enter_context(tc.tile_pool(name="scp", bufs=2))
            scratch = ctx.enter_context(tc.tile_pool(name="scratch", bufs=4))
            scaledp = ctx.enter_context(tc.tile_pool(name="scaledp", bufs=8))
            outp = ctx.enter_context(tc.tile_pool(name="outp", bufs=2))
            psp = ctx.enter_context(tc.tile_pool(name="psp", bufs=2, space="PSUM"))
            psq = ctx.enter_context(tc.tile_pool(name="psq", bufs=3, space="PSUM"))
            pss = ctx.enter_context(tc.tile_pool(name="pss", bufs=1, space="PSUM"))

            # ---- one-time setup ----
            identity = consts.tile([128, 128], f32)
            nc.sync.dma_start(identity[:], ident_dram[:])
            id_bf = consts.tile([128, 128], bf16)
            nc.scalar.copy(id_bf[:], identity[:])

            # wT = 0.125 * W_key^T  as [64, 132]
            w1 = consts.tile([128, DQ], f32)
            nc.sync.dma_start(w1[:], wk[0:128, :])
            w2 = consts.tile([4, DQ], f32)
            nc.sync.dma_start(w2[:], wk[128:132, :])
            wT = consts.tile([DQ, F], f32)
            stp = pss.tile([DQ, 128], f32, tag="setup_ps")
            nc.tensor.transpose(stp[:], w1[:], identity[:])
            nc.scalar.mul(wT[:, 0:128], stp[:], 0.125)
            stp2 = pss.tile([DQ, 4], f32, tag="setup_ps")
            nc.tensor.transpose(stp2[:], w2[:], identity[0:4, 0:4])
            nc.scalar.mul(wT[:, 128:132], stp2[:], 0.125)

            def emit_loads(gi):
                """DMA fp32 near chunks + delta tensors for group gi."""
                b0 = gi * G
                dxy_t = dlp.tile([G, N, 2], f32, tag="dl")
                nc.sync.dma_start(dxy_t[:], dxy[b0 : b0 + G, :, :])
                dcs_t = dlp.tile([G, N, 2], f32, tag="dl")
                nc.sync.dma_start(dcs_t[:], dcs[b0 : b0 + G, :, :])
                nmf = []
                for c in range(NT):
                    t = nearf.tile([G, NCH, D], f32, name=f"nf{gi}_{c}", tag="nf")
                    nc.sync.dma_start(
                        t[:], near[b0 : b0 + G, c * NCH : (c + 1) * NCH, :]
                    )
                    nmf.append(t)
                return dxy_t, dcs_t, nmf

            def emit_qp(gi):
                b0 = gi * G
                bq_t = smq.tile([G, DQ], f32, tag="sm")
                nc.sync.dma_start(bq_t[:], bq[b0 : b0 + G, :])
                bqT_ps = psq.tile([DQ, G], f32, tag="qpps")
                nc.tensor.transpose(bqT_ps[:], bq_t[:], identity[:])
                bqT = smq.tile([DQ, G], f32, tag="sm")
                nc.scalar.copy(bqT[:], bqT_ps[:])
                qp_ps = psq.tile([G, F], f32, tag="qpps")
                nc.tensor.matmul(qp_ps[:], bqT[:], wT[:], start=True, stop=True)
                qp = qpp.tile([G, F], f32, tag="qp")
                nc.scalar.copy(qp[:], qp_ps[:])
                return qp

            qp = emit_qp(0)
            loads = emit_loads(0)
            pending = None

            def emit_epilogue(p_pooled4, p_recip, p_b0):
                # out = (band0 + band1) / sumexp, one PSUM operand per op:
                # a0 = band0 * recip; out = (band1 * recip) + a0
                a0 = scratch.tile([G, D], f32, tag="a0")
                nc.vector.tensor_scalar_mul(a0[:], p_pooled4[:, 0, :], p_recip[:])
                out_t = outp.tile([G, D], f32, tag="out")
                nc.vector.scalar_tensor_tensor(
                    out=out_t[:],
                    in0=p_pooled4[:, 1, :],
                    scalar=p_recip[:],
                    in1=a0[:],
                    op0=mult,
                    op1=add,
                )
                nc.sync.dma_start(out[p_b0 : p_b0 + G, :], out_t[:])

            for gi in range(NGROUPS):
                b0 = gi * G
                dxy_t, dcs_t, nmf = loads
                if gi + 1 < NGROUPS:
                    qp_next = emit_qp(gi + 1)
                    loads = emit_loads(gi + 1)
                else:
                    qp_next = None

                # ---- delta score contribution sc4[g, n] (DVE, fp32) ----
                s1 = scp.tile([G, N], f32, tag="s1")
                nc.vector.tensor_scalar_mul(s1[:], dxy_t[:, :, 0], qp[:, 128:129])
                s2 = scp.tile([G, N], f32, tag="s2")
                nc.vector.scalar_tensor_tensor(
                    s2[:], dxy_t[:, :, 1], qp[:, 129:130], s1[:], op0=mult, op1=add
                )
                s3 = scp.tile([G, N], f32, tag="s3")
                nc.vector.scalar_tensor_tensor(
                    s3[:], dcs_t[:, :, 0], qp[:, 130:131], s2[:], op0=mult, op1=add
                )
                sc4 = scp.tile([G, N], f32, tag="sc4")
                nc.vector.scalar_tensor_tensor(
                    sc4[:], dcs_t[:, :, 1], qp[:, 131:132], s3[:], op0=mult, op1=add
                )

                # ---- segmented: scores -> exp -> bf16 quads -> matmul ----
                # Fine segments for the very first block (pipeline fill);
                # half-group segments otherwise (fewer, wider add/exp ops).
                scores0 = scp.tile([G, N], f32, tag="scores0")
                scsum = scp.tile([G, N], f32, tag="scsum")
                e_t = scp.tile([G, N], f32, tag="et")
                pooled4 = psp.tile([G, NQ, D], f32, tag="pool")
                if gi == 0:
                    segs = [(0, 16), (16, 32), (32, 48), (48, 64),
                            (64, 96), (96, 128)]
                elif gi == 1:
                    segs = [(0, 32), (32, 64), (64, 128)]
                else:
                    segs = [(0, 64), (64, 128)]
                for n0, n1 in segs:
                    ss = slice(n0, n1)
                    for n in range(n0, n1):
                        c, j = divmod(n, NCH)
                        pr = scratch.tile([G, D], f32, name=f"pr{n}", tag="pr")
                        nc.vector.scalar_tensor_tensor(
                            out=pr[:],
                            in0=nmf[c][:, j, :],
                            scalar=1.0,
                            in1=qp[:, 0:D],
                            op0=bypass,
                            op1=mult,
                            accum_out=scores0[:, n : n + 1],
                        )
                    nc.vector.tensor_tensor(
                        scsum[:, ss], scores0[:, ss], sc4[:, ss], op=add
                    )
                    nc.scalar.activation(
                        e_t[:, ss],
                        scsum[:, ss],
                        func=mybir.ActivationFunctionType.Exp,
                    )
                    for q in range(n0 // NQ, n1 // NQ):
                        quad = scaledp.tile([G, NQ, D], bf16, name=f"qd{q}", tag="qd")
                        for k in range(NQ):
                            n = q * NQ + k
                            c, j = divmod(n, NCH)
                            eng = pattern[n % len(pattern)]
                            eap = e_t[:, n : n + 1]
                            if eng == "a":
                                nc.scalar.mul(quad[:, k, :], nmf[c][:, j, :], eap)
                            else:
                                nc.vector.tensor_scalar_mul(
                                    quad[:, k, :], nmf[c][:, j, :], eap
                                )
                        nc.tensor.matmul(
                            pooled4[:],
                            id_bf[:],
                            quad[:],
                            start=(q == 0),
                            stop=(q == N // NQ - 1),
                        )

                # ---- sumexp, reciprocal ----
                sume = scp.tile([G, 1], f32, tag="sume")
                nc.vector.tensor_reduce(
                    out=sume[:], in_=e_t[:], axis=mybir.AxisListType.X, op=add
                )
                recip = scp.tile([G, 1], f32, tag="recip")
                nc.vector.reciprocal(recip[:], sume[:])

                # ---- deferred epilogue of previous group ----
                if pending is not None:
                    emit_epilogue(*pending)
                pending = (pooled4, recip, b0)
                qp = qp_next

            emit_epilogue(*pending)

    nc.compile()
    return nc


def _get_nc():
    global _NC
    if _NC is None:
        _NC = _build()
    return _NC


def kernel(near_emb, delta_xy, delta_cs, B_query, W_key, b_key=None, **_ignored):
    from concourse import bass_utils

    near_emb = np.ascontiguousarray(np.asarray(near_emb, dtype=np.float32))
    delta_xy = np.ascontiguousarray(np.asarray(delta_xy, dtype=np.float32))
    delta_cs = np.ascontiguousarray(np.asarray(delta_cs, dtype=np.float32))
    B_query = np.ascontiguousarray(np.asarray(B_query, dtype=np.float32))
    W_key = np.ascontiguousarray(np.asarray(W_key, dtype=np.float32))

    nc = _get_nc()
    in_maps = []
    for c in range(CORES):
        s = slice(c * PB, (c + 1) * PB)
        in_maps.append(
            {
                "near": near_emb[s],
                "dxy": delta_xy[s],
                "dcs": delta_cs[s],
                "bq": B_query[s],
                "wk": W_key,
            }
        )
    res = bass_utils.run_bass_kernel_spmd(nc, in_maps, core_ids=list(range(CORES)))
    return np.concatenate([res.results[c]["out"] for c in range(CORES)], axis=0)


# revision 30
# speedup vs baseline: 1.1117x; 1.0161x over previous
"""NearAggregator Trainium2 Bass kernel.

Math (per batch item b):
    Kcat   = concat([near_emb, delta_xy, delta_cs], -1)          # [N, 132]
    scores = (Kcat @ W_key + b_key) . B_query[b] / sqrt(64)      # [N]
    out[b] = softmax(scores) @ near_emb[b]                       # [128]

Reformulation:
  * Fold W_key into the query side:  qp[b,:] = 0.125 * (W_key^T @ B_query[b])
    (132-dim), so scores[b,n] = near[b,n,:].qp[b,:128] + delta[b,n,:].qp[b,128:132].
  * b_key only shifts scores by a per-b constant -> softmax-invariant -> dropped.
  * softmax without max-subtraction: scores ~ N(0, 0.58), exp() safe in fp32.

Pipeline (per 128-item group g, engines in parallel):
  * DVE computes scores via fused fp32 mult+reduce (near_n . qp) per
    neighbor; delta contribution via 4 fp32 ops; exp per 16-neighbor chunk
    on ScalarE (chunk-local softmax needs no max subtraction); 1/sumexp on
    DVE once per group.
  * scaled_n = near_n * e[:,n] -> bf16 (inline convert), split
    ScalarE/DVE by a tunable mix. Four neighbors' scaled tiles share one
    [G, 4, D] quad tile.
  * TensorE accumulates each quad with a single matmul (identity_bf16
    stationary, 512-wide moving) into a [G, 4, D] PSUM accumulator:
    4x fewer PE instructions, no per-neighbor weight reload stalls.
  * Epilogue (deferred one group): sum the 4 PSUM bands + multiply by
    1/sumexp on DVE, store.

Data parallel over 8 NeuronCores: batch 8192 -> 1024 per core.
"""

import os

import numpy as np

B = 8192
N = 128
D = 128
DQ = 64
F = D + 4
CORES = 8
PB = B // CORES            # 1024 items per core
G = 128                    # items per group (= partition dim)
NGROUPS = PB // G          # 8
NCH = 16                   # neighbors per near tile
NT = N // NCH              # 8 tiles per group
NQ = 2                     # neighbors per matmul quad


# Per-neighbor scale-op engine mix: a=ScalarE, v=DVE (bf16 fast path).
def _mix_pattern(spec):
    parts = [(k, int(v)) for k, v in (p.split(":") for p in spec.split(","))]
    total = sum(c for _, c in parts)
    pat = []
    err = [0.0] * len(parts)
    for _ in range(total):
        for i in range(len(parts)):
            err[i] += parts[i][1] / total
        best = max(range(len(parts)), key=lambda i: err[i])
        err[best] -= 1.0
        pat.append(parts[best][0])
    return "".join(pat)


SCALE_PATTERN = _mix_pattern(os.environ.get("NK_MIX", "a:7,v:3"))
# Last group has no following work to overlap; finish both engines together.
DRAIN_PATTERN = _mix_pattern(os.environ.get("NK_DRAIN_MIX", "a:1,v:1"))

_NC = None


def _build():
    import concourse.tile as tile
    from concourse import bacc, mybir

    f32 = mybir.dt.float32
    bf16 = mybir.dt.bfloat16
    mult = mybir.AluOpType.mult
    add = mybir.AluOpType.add
    bypass = mybir.AluOpType.bypass

    nc = bacc.Bacc(
        "TRN2",
        target_bir_lowering=False,
        debug=False,
        enable_asserts=True,
        num_devices=CORES,
    )
    near = nc.dram_tensor("near", [PB, N, D], f32, kind="ExternalInput").ap()
    dxy = nc.dram_tensor("dxy", [PB, N, 2], f32, kind="ExternalInput").ap()
    dcs = nc.dram_tensor("dcs", [PB, N, 2], f32, kind="ExternalInput").ap()
    bq = nc.dram_tensor("bq", [PB, DQ], f32, kind="ExternalInput").ap()
    wk = nc.dram_tensor("wk", [F, DQ], f32, kind="ExternalInput").ap()
    out = nc.dram_tensor("out", [PB, D], f32, kind="ExternalOutput").ap()
    ident_dram = nc.inline_tensor(np.eye(128, dtype=np.float32), name="ident").ap()

    pattern = SCALE_PATTERN

    with tile.TileContext(nc) as tc:
        from contextlib import ExitStack

        ctx = ExitStack()
        with ctx:
            consts = ctx.enter_context(tc.tile_pool(name="consts", bufs=1))
            nearf = ctx.enter_context(tc.tile_pool(name="nearf", bufs=2 * NT + 1))
            dlp = ctx.enter_context(tc.tile_pool(name="dlp", bufs=4))
            smq = ctx.enter_context(tc.tile_pool(name="smq", bufs=4))
            qpp = ctx.enter_context(tc.tile_pool(name="qpp", bufs=2))
            scp = ctx.enter_context(tc.tile_pool(name="scp", bufs=2))
            scratch = ctx.enter_context(tc.tile_pool(name="scratch", bufs=4))
            scaledp = ctx.enter_context(tc.tile_pool(name="scaledp", bufs=8))
            outp = ctx.enter_context(tc.tile_pool(name="outp", bufs=2))
            psp = ctx.enter_context(tc.tile_pool(name="psp", bufs=2, space="PSUM"))
            psq = ctx.enter_context(tc.tile_pool(name="psq", bufs=3, space="PSUM"))
            pss = ctx.enter_context(tc.tile_pool(name="pss", bufs=1, space="PSUM"))

            # ---- one-time setup ----
            identity = consts.tile([128, 128], f32)
            nc.sync.dma_start(identity[:], ident_dram[:])
            id_bf = consts.tile([128, 128], bf16)
            nc.scalar.copy(id_bf[:], identity[:])

            # wT = 0.125 * W_key^T  as [64, 132]
            w1 = consts.tile([128, DQ], f32)
            nc.sync.dma_start(w1[:], wk[0:128, :])
            w2 = consts.tile([4, DQ], f32)
            nc.sync.dma_start(w2[:], wk[128:132, :])
            wT = consts.tile([DQ, F], f32)
            stp = pss.tile([DQ, 128], f32, tag="setup_ps")
            nc.tensor.transpose(stp[:], w1[:], identity[:])
            nc.scalar.mul(wT[:, 0:128], stp[:], 0.125)
            stp2 = pss.tile([DQ, 4], f32, tag="setup_ps")
            nc.tensor.transpose(stp2[:], w2[:], identity[0:4, 0:4])
            nc.scalar.mul(wT[:, 128:132], stp2[:], 0.125)

            def emit_loads(gi):
                """DMA fp32 near chunks + delta tensors for group gi."""
                b0 = gi * G
                dxy_t = dlp.tile([G, N, 2], f32, tag="dl")
                nc.sync.dma_start(dxy_t[:], dxy[b0 : b0 + G, :, :])
                dcs_t = dlp.tile([G, N, 2], f32, tag="dl")
                nc.sync.dma_start(dcs_t[:], dcs[b0 : b0 + G, :, :])
                nmf = []
                for c in range(NT):
                    t = nearf.tile([G, NCH, D], f32, name=f"nf{gi}_{c}", tag="nf")
                    nc.sync.dma_start(
                        t[:], near[b0 : b0 + G, c * NCH : (c + 1) * NCH, :]
                    )
                    nmf.append(t)
                return dxy_t, dcs_t, nmf

            def emit_qp(gi):
                b0 = gi * G
                bq_t = smq.tile([G, DQ], f32, tag="sm")
                nc.sync.dma_start(bq_t[:], bq[b0 : b0 + G, :])
                bqT_ps = psq.tile([DQ, G], f32, tag="qpps")
                nc.tensor.transpose(bqT_ps[:], bq_t[:], identity[:])
                bqT = smq.tile([DQ, G], f32, tag="sm")
                nc.scalar.copy(bqT[:], bqT_ps[:])
                qp_ps = psq.tile([G, F], f32, tag="qpps")
                nc.tensor.matmul(qp_ps[:], bqT[:], wT[:], start=True, stop=True)
                qp = qpp.tile([G, F], f32, tag="qp")
                nc.scalar.copy(qp[:], qp_ps[:])
                return qp

            qp = emit_qp(0)
            loads = emit_loads(0)
            pending = None

            def emit_epilogue(p_pooled4, p_recip, p_b0):
                # out = (band0 + band1) / sumexp, one PSUM operand per op:
                # a0 = band0 * recip; out = (band1 * recip) + a0
                a0 = scratch.tile([G, D], f32, tag="a0")
                nc.vector.tensor_scalar_mul(a0[:], p_pooled4[:, 0, :], p_recip[:])
                out_t = outp.tile([G, D], f32, tag="out")
                nc.vector.scalar_tensor_tensor(
                    out=out_t[:],
                    in0=p_pooled4[:, 1, :],
                    scalar=p_recip[:],
                    in1=a0[:],
                    op0=mult,
                    op1=add,
                )
                nc.sync.dma_start(out[p_b0 : p_b0 + G, :], out_t[:])

            for gi in range(NGROUPS):
                b0 = gi * G
                dxy_t, dcs_t, nmf = loads
                if gi + 1 < NGROUPS:
                    qp_next = emit_qp(gi + 1)
                    loads = emit_loads(gi + 1)
                else:
                    qp_next = None

                # ---- delta score contribution sc4[g, n] (DVE, fp32) ----
                s1 = scp.tile([G, N], f32, tag="s1")
                nc.vector.tensor_scalar_mul(s1[:], dxy_t[:, :, 0], qp[:, 128:129])
                s2 = scp.tile([G, N], f32, tag="s2")
                nc.vector.scalar_tensor_tensor(
                    s2[:], dxy_t[:, :, 1], qp[:, 129:130], s1[:], op0=mult, op1=add
                )
                s3 = scp.tile([G, N], f32, tag="s3")
                nc.vector.scalar_tensor_tensor(
                    s3[:], dcs_t[:, :, 0], qp[:, 130:131], s2[:], op0=mult, op1=add
                )
                sc4 = scp.tile([G, N], f32, tag="sc4")
                nc.vector.scalar_tensor_tensor(
                    sc4[:], dcs_t[:, :, 1], qp[:, 131:132], s3[:], op0=mult, op1=add
                )

                # ---- segmented: scores -> exp -> bf16 quads -> matmul ----
                # Fine segments for the very first block (pipeline fill);
                # half-group segments otherwise (fewer, wider add/exp ops).
                scores0 = scp.tile([G, N], f32, tag="scores0")
                scsum = scp.tile([G, N], f32, tag="scsum")
                e_t = scp.tile([G, N], f32, tag="et")
                pooled4 = psp.tile([G, NQ, D], f32, tag="pool")
                if gi == 0:
                    segs = [(0, 16), (16, 32), (32, 48), (48, 64),
                            (64, 96), (96, 128)]
                elif gi == 1:
                    segs = [(0, 32), (32, 64), (64, 128)]
                else:
                    segs = [(0, 64), (64, 128)]
                for n0, n1 in segs:
                    ss = slice(n0, n1)
                    for n in range(n0, n1):
                        c, j = divmod(n, NCH)
                        pr = scratch.tile([G, D], f32, name=f"pr{n}", tag="pr")
                        nc.vector.scalar_tensor_tensor(
                            out=pr[:],
                            in0=nmf[c][:, j, :],
                            scalar=1.0,
                            in1=qp[:, 0:D],
                            op0=bypass,
                            op1=mult,
                            accum_out=scores0[:, n : n + 1],
                        )
                    nc.vector.tensor_tensor(
                        scsum[:, ss], scores0[:, ss], sc4[:, ss], op=add
                    )
                    nc.scalar.activation(
                        e_t[:, ss],
                        scsum[:, ss],
                        func=mybir.ActivationFunctionType.Exp,
                    )
                    pat_g = DRAIN_PATTERN if gi == NGROUPS - 1 else pattern
                    for q in range(n0 // NQ, n1 // NQ):
                        quad = scaledp.tile([G, NQ, D], bf16, name=f"qd{q}", tag="qd")
                        eng = pat_g[q % len(pat_g)]
                        if eng == "v":
                            # one DVE broadcast-mult scales the whole pair
                            c, j = divmod(q * NQ, NCH)
                            eb = e_t[:, q * NQ : q * NQ + NQ].unsqueeze(
                                2
                            ).broadcast_to((G, NQ, D))
                            nc.vector.tensor_tensor(
                                quad[:], nmf[c][:, j : j + NQ, :], eb, op=mult
                            )
                        else:
                            for k in range(NQ):
                                n = q * NQ + k
                                c, j = divmod(n, NCH)
                                eap = e_t[:, n : n + 1]
                                nc.scalar.mul(quad[:, k, :], nmf[c][:, j, :], eap)
                        nc.tensor.matmul(
                            pooled4[:],
                            id_bf[:],
                            quad[:],
                            start=(q == 0),
                            stop=(q == N // NQ - 1),
                        )

                # ---- sumexp, reciprocal ----
                sume = scp.tile([G, 1], f32, tag="sume")
                nc.vector.tensor_reduce(
                    out=sume[:], in_=e_t[:], axis=mybir.AxisListType.X, op=add
                )
                recip = scp.tile([G, 1], f32, tag="recip")
                nc.vector.reciprocal(recip[:], sume[:])

                # ---- deferred epilogue of previous group ----
                if pending is not None:
                    emit_epilogue(*pending)
                pending = (pooled4, recip, b0)
                qp = qp_next

            emit_epilogue(*pending)

    nc.compile()
    return nc


def _get_nc():
    global _NC
    if _NC is None:
        _NC = _build()
    return _NC


def kernel(near_emb, delta_xy, delta_cs, B_query, W_key, b_key=None, **_ignored):
    from concourse import bass_utils

    near_emb = np.ascontiguousarray(np.asarray(near_emb, dtype=np.float32))
    delta_xy = np.ascontiguousarray(np.asarray(delta_xy, dtype=np.float32))
    delta_cs = np.ascontiguousarray(np.asarray(delta_cs, dtype=np.float32))
    B_query = np.ascontiguousarray(np.asarray(B_query, dtype=np.float32))
    W_key = np.ascontiguousarray(np.asarray(W_key, dtype=np.float32))

    nc = _get_nc()
    in_maps = []
    for c in range(CORES):
        s = slice(c * PB, (c + 1) * PB)
        in_maps.append(
            {
                "near": near_emb[s],
                "dxy": delta_xy[s],
                "dcs": delta_cs[s],
                "bq": B_query[s],
                "wk": W_key,
            }
        )
    res = bass_utils.run_bass_kernel_spmd(nc, in_maps, core_ids=list(range(CORES)))
    return np.concatenate([res.results[c]["out"] for c in range(CORES)], axis=0)
